# revision 8
# baseline (speedup 1.0000x reference)
"""Trainium2 Bass kernel for nn_Melody_RNN (B=64, S=512, A=20, V=130, E=H=64, L=2).

Structure exploited (all implied by the reference's exact semantics):
  * Only embedding rows for inputs[:,0] / inputs[:,1] are used; the LSTM runs
    exactly 2 timesteps (zero initial state, so the forget gate is dead).
  * The torch cat+view memory reinterpretations make every output row a
    function of s only: row s = OG[s] for s<84, OG[20+(s-84)%64] for s>=84,
    identical across batches except batch 0 rows 0..83 (OB table).
  * The attention-mask bug makes softmax exactly uniform, so the sliding
    window sum is a fixed linear map: attn = whTT^T @ Ch + wcTT^T @ Cc + bias
    with host-precomputed integer count matrices C.

Kernel v6 design:
  * bf16 LSTM batched over both timesteps (4 matmuls + 8 ACTs + 4 muls).
  * Window/attention collapsed into matmuls against host count matrices
    (bf16; counts are small ints, exact).
  * bf16 decode -> og/ob row tables [96,130] (rows 84:96 = generic rows
    20:32, produced by widening the decode matmul) -> DRAM staging
    (SBUF APs cannot fold partitions) -> Y [6,4160] = flattened 32-row
    classes (H=og[0:32], A=og[32:64], B=og[64:96]; H'/A'/B' from ob).
  * Selection matmul (bf16 0/1 weights, exact) broadcasts Y -> X [128,4160]
    in 512-col PSUM chunks; DVE/ACT copy to SBUF as f32.
  * X partition p = output row-block p (slot-major p = 16m+q), so the
    output is X flattened: 3 column-wave DMAs of 128 descriptors x 5-6KB
    each saturate all 16 SDMA engines near the HBM write roofline.

SPMD: 8 cores identical program; core 0's input pack carries the batch-0
count matrices and mvec=1 so its slot 0 blends to the special table.
"""

import sys
import numpy as np

if "/root/.axon_site/_ro/trn_rl_repo" not in sys.path:
    sys.path.insert(0, "/root/.axon_site/_ro/trn_rl_repo")

B, S, A = 64, 512, 20
V, E, H = 130, 64, 64
NCORES = 8
BPC = B // NCORES  # batches per core

SLOT = S * V          # 66560 elements per slot
RG = 32 * V           # 4160 elements per row-group / X partition

# aux bf16 pack columns
_XS = 0
_WIH0 = 128
_WIH1 = 320
_WHW = 512
_WCW = 640
_CBH = 768
_CBC = 852
_CGH = 936
_CGC = 1020
_CNT = 1104   # partition 0, [1, 84]
_WHBR = 1188  # partition 0, [1, 64]
_SEL = 1252   # partitions 0:6, [6, 128]
_DECA = 1380  # [64, 130]
_DECW2 = 1510  # [65, 130]
W2 = 1640
W1 = 8        # f32 bias pack cols

_NC_CACHE = {}


def _build_nc():
    import concourse.bass as bass
    import concourse.bacc as bacc
    import concourse.mybir as mybir
    from concourse.tile import TileContext

    f32 = mybir.dt.float32
    bf16 = mybir.dt.bfloat16
    AF = mybir.ActivationFunctionType

    nc = bacc.Bacc("TRN2", target_bir_lowering=False, debug=False)

    d_bias = nc.dram_tensor("biasp", [128, W1], f32, kind="ExternalInput")
    d_aux = nc.dram_tensor("aux", [128, W2], bf16, kind="ExternalInput")
    d_out = nc.dram_tensor("out", [BPC * S, V], f32, kind="ExternalOutput")
    d_fold = nc.dram_tensor("fold", [2, 96 * V], bf16, kind="Internal")

    with TileContext(nc) as tc:
        with (
            tc.tile_pool(name="sbuf", bufs=1) as pool,
            tc.tile_pool(name="psum", bufs=1, space="PSUM") as pp,
            tc.tile_pool(name="psx", bufs=3, space="PSUM") as px,
        ):
            # ---- input loads: LSTM-critical slice first ----
            t_aux = pool.tile([128, W2], bf16)
            t_bias = pool.tile([128, W1], f32)
            nc.sync.dma_start(out=t_aux[:, 0:512], in_=d_aux[:, 0:512])
            nc.scalar.dma_start(out=t_aux[:, 512:W2], in_=d_aux[:, 512:W2])
            nc.sync.dma_start(out=t_bias[:], in_=d_bias[:])

            xs = t_aux[0:64, _XS:_XS + 128]
            wih0 = t_aux[0:64, _WIH0:_WIH0 + 192]
            wih1 = t_aux[0:64, _WIH1:_WIH1 + 192]
            whw = t_aux[0:64, _WHW:_WHW + 128]
            wcw = t_aux[0:64, _WCW:_WCW + 128]
            BIA = t_bias

            ones = pool.tile([1, 4], f32)
            dummy = pool.tile([1, 2], f32)
            nc.vector.memset(ones[:], 1.0)
            # preload Sigmoid/Tanh ACT tables while input DMAs are in flight
            nc.scalar.activation(dummy[0:1, 0:1], ones[0:1, 0:1], AF.Sigmoid)
            nc.scalar.activation(dummy[0:1, 1:2], ones[0:1, 0:1], AF.Tanh)

            # ---- LSTM: both steps batched; hcat cols [l0s0|l0s1|l1s0|l1s1]
            hcat = pool.tile([H, 256], bf16)
            ccat = pool.tile([H, 256], bf16)

            def lstm_layer(rhsT, wp, bc, dst_off, tag):
                psg = pp.tile([128, 256], f32, tag="gates")
                nc.tensor.matmul(psg[:, 0:128], wp[:, 0:128], rhsT,
                                 start=True, stop=True)
                nc.tensor.matmul(psg[0:64, 128:256], wp[:, 128:192], rhsT,
                                 start=True, stop=True)
                sig_i = pool.tile([H, 128], f32, tag=f"sigi{tag}")
                tanh_g = pool.tile([H, 128], f32, tag=f"tanhg{tag}")
                sig_o = pool.tile([H, 128], f32, tag=f"sigo{tag}")
                tanh_c = pool.tile([H, 128], f32, tag=f"tanhc{tag}")
                nc.scalar.activation(tanh_g[:], psg[64:128, 0:128], AF.Tanh,
                                     bias=BIA[64:128, bc:bc + 1])
                nc.scalar.activation(sig_i[:], psg[0:64, 0:128], AF.Sigmoid,
                                     bias=BIA[0:64, bc:bc + 1])
                nc.scalar.activation(sig_o[:], psg[0:64, 128:256], AF.Sigmoid,
                                     bias=BIA[0:64, bc + 1:bc + 2])
                cc = ccat[:, dst_off:dst_off + 128]
                hh = hcat[:, dst_off:dst_off + 128]
                nc.vector.tensor_mul(cc, sig_i[:], tanh_g[:])
                nc.scalar.activation(tanh_c[:], cc, AF.Tanh)
                nc.vector.tensor_mul(hh, sig_o[:], tanh_c[:])

            lstm_layer(xs, wih0, 0, 0, "l0")
            lstm_layer(hcat[:, 0:128], wih1, 2, 128, "l1")
            out0T = hcat[:, 128:192]
            out1T = hcat[:, 192:256]

            # ---- outputs-half tiles [65, 96] bf16 (row 64 = ones) ----
            # cols 84:96 repeat generic rows 20:32 (periodic pad for the fold)
            outG = pool.tile([65, 96], bf16)
            outB = pool.tile([65, 96], bf16)
            d64 = pool.tile([64, 64], f32)
            MV = BIA[0:64, 6:7]
            nc.vector.tensor_sub(d64[:], out0T, out1T)
            nc.vector.tensor_scalar_mul(d64[:], d64[:], MV)
            nc.vector.tensor_add(outB[0:64, 0:64], out1T, d64[:])
            nc.vector.tensor_copy(outB[0:64, 64:96], out1T[:, 0:32])
            nc.vector.memset(outB[64:65, :], 1.0)
            nc.gpsimd.tensor_copy(outG[0:64, 0:64], out1T)
            nc.gpsimd.tensor_copy(outG[0:64, 64:96], out1T[:, 0:32])
            nc.vector.memset(outG[64:65, :], 1.0)

            # ---- mm1: whTT/wcTT [128, 64] (bf16 copies for mm2) ----
            pstab = pp.tile([128, 128], f32, tag="tab")
            whTTp = pstab[:, 0:64]
            wcTTp = pstab[:, 64:128]
            nc.tensor.matmul(whTTp, hcat[:, 0:256:2], whw[:, 0:64],
                             start=True, stop=False)
            nc.tensor.matmul(whTTp, hcat[:, 1:256:2], whw[:, 64:128],
                             start=False, stop=True)
            nc.tensor.matmul(wcTTp, ccat[:, 0:256:2], wcw[:, 0:64],
                             start=True, stop=False)
            nc.tensor.matmul(wcTTp, ccat[:, 1:256:2], wcw[:, 64:128],
                             start=False, stop=True)
            whTT = pool.tile([128, 64], bf16)
            wcTT = pool.tile([128, 64], bf16)
            nc.scalar.copy(whTT[:], whTTp)
            nc.vector.tensor_copy(wcTT[:], wcTTp)

            # ---- mm2: attn [64, 84] = whTT^T@Ch + wcTT^T@Cc + whb x cntfix
            CBh = t_aux[:, _CBH:_CBH + 84]
            CBc = t_aux[:, _CBC:_CBC + 84]
            CGh = t_aux[:, _CGH:_CGH + 84]
            CGc = t_aux[:, _CGC:_CGC + 84]
            CNT = t_aux[0:1, _CNT:_CNT + 84]
            WHBR = t_aux[0:1, _WHBR:_WHBR + 64]
            ABIAS = BIA[0:64, 4:5]
            psat = pp.tile([64, 168], f32, tag="attn")

            def attn_mm(Ch, Cc, c0):
                ps = psat[:, c0:c0 + 84]
                nc.tensor.matmul(ps, whTT[:], Ch, start=True, stop=False)
                nc.tensor.matmul(ps, wcTT[:], Cc, start=False, stop=False)
                nc.tensor.matmul(ps, WHBR, CNT, start=False, stop=True)
                return ps

            attnBp = attn_mm(CBh, CBc, 84)
            attnGp = attn_mm(CGh, CGc, 0)
            attnG = pool.tile([64, 96], bf16)
            attnB = pool.tile([64, 96], bf16)
            nc.vector.tensor_scalar_add(attnB[:, 0:84], attnBp, ABIAS)
            nc.vector.tensor_scalar_add(attnB[:, 84:96], psat[0:64, 20:32], ABIAS)
            nc.scalar.activation(attnG[:, 0:84], attnGp, AF.Identity, bias=ABIAS)
            nc.scalar.activation(attnG[:, 84:96], psat[0:64, 20:32],
                                 AF.Identity, bias=ABIAS)

            # ---- decode: ob first (its fold gates the output pipeline) ----
            DECA = t_aux[0:64, _DECA:_DECA + 130]
            DECW2 = t_aux[0:65, _DECW2:_DECW2 + 130]
            psdec = pp.tile([96, 2 * V], f32, tag="dec")
            obPp = psdec[:, 0:V]
            ogPp = psdec[:, V:2 * V]
            nc.tensor.matmul(obPp, outB[:], DECW2, start=True, stop=False)
            nc.tensor.matmul(obPp, attnB[:], DECA, start=False, stop=True)
            nc.tensor.matmul(ogPp, outG[:], DECW2, start=True, stop=False)
            nc.tensor.matmul(ogPp, attnG[:], DECA, start=False, stop=True)

            og96 = pool.tile([96, V], bf16)
            ob96 = pool.tile([96, V], bf16)
            nc.vector.tensor_copy(ob96[:], obPp)
            nc.scalar.copy(og96[:], ogPp)

            # ---- fold via DRAM: og96/ob96 -> d_fold -> Y [6, 4160] ----
            # (SBUF APs may only cross partitions in dim 0, so a direct
            # SBUF->SBUF fold is not expressible; DRAM staging is.)
            Y = pool.tile([6, RG], bf16)
            Yt = Y[:].tensor
            nc.sync.dma_start(out=d_fold[1:2, :], in_=ob96[:])
            nc.sync.dma_start(out=d_fold[0:1, :], in_=og96[:])
            for c0, cw in ((0, 2080), (2080, 2080)):
                nc.sync.dma_start(
                    out=bass.AP(Yt, c0, [[RG, 6], [1, cw]]),
                    in_=bass.AP(d_fold, c0, [[RG, 6], [1, cw]]))

            # ---- X broadcast: Sel [6,128] x Y chunks -> X [128, 4160] ----
            SEL = t_aux[0:6, _SEL:_SEL + 128]
            X = pool.tile([128, RG], f32)
            Xt = X[:].tensor
            copy_eng = [nc.vector, nc.scalar]
            CHUNKS = [(n * 512, min(512, RG - n * 512)) for n in range(9)]

            def x_chunks(rng):
                for n in rng:
                    c0, cw = CHUNKS[n]
                    ps = px.tile([128, 512], f32, tag="xch")
                    nc.tensor.matmul(ps[:, 0:cw], SEL, Y[:, c0:c0 + cw],
                                     start=True, stop=True)
                    eng = copy_eng[n % 2]
                    if eng is nc.vector:
                        eng.tensor_copy(X[:, c0:c0 + cw], ps[:, 0:cw])
                    else:
                        eng.copy(X[:, c0:c0 + cw], ps[:, 0:cw])

            def out_wave(col0, cw):
                # X partition p = output row-block p: pure partition-major
                nc.gpsimd.dma_start(
                    out=bass.AP(d_out, col0, [[RG, 128], [1, cw]]),
                    in_=bass.AP(Xt, col0, [[RG, 128], [1, cw]]))

            x_chunks(range(0, 3))
            out_wave(0, 1536)
            x_chunks(range(3, 6))
            out_wave(1536, 1536)
            x_chunks(range(6, 9))
            out_wave(3072, RG - 3072)

    nc.compile()
    return nc


def _get_nc():
    if "nc" not in _NC_CACHE:
        _NC_CACHE["nc"] = _build_nc()
    return _NC_CACHE["nc"]


def _build_count_matrices():
    colmapG = list(range(32, 64)) + list(range(96, 128)) + list(range(32, 51))
    colmap0 = list(range(0, 32)) + list(range(64, 96)) + list(range(32, 51))
    wccolG = list(range(32, 64)) + list(range(96, 128)) + list(range(32, 52))
    wccol0 = list(range(0, 32)) + list(range(64, 96)) + list(range(32, 52))

    def ch(colmap):
        C = np.zeros((128, 84), np.float32)
        cnt = np.zeros(84, np.float32)
        for r in range(84):
            for j in range(max(r, 20), r + 20):
                C[colmap[j - 20], r] += 1.0
                cnt[r] += 1.0
        return C, cnt

    def cc(wccol):
        C = np.zeros((128, 84), np.float32)
        for r in range(84):
            C[wccol[r], r] += 1.0
        return C

    CGh, cnt = ch(colmapG)
    C0h, _ = ch(colmap0)
    cntfix = (cnt - 20.0) / A
    return CGh, cc(wccolG), C0h, cc(wccol0), cntfix


def _host_reference_fallback(inputs):
    """Pure-numpy replica of the reference for steps != 512 (never hit with the
    canonical setup_inputs, which fixes lengths = 512)."""
    emb = inputs["emb"]; L = 2
    Ls = np.asarray(inputs["lengths"]); steps = int(Ls.max()); batch = inputs["inputs"].shape[0]
    layers = [(inputs["Wih0"], inputs["bih0"], inputs["bhh0"]),
              (inputs["Wih1"], inputs["bih1"], inputs["bhh1"])]
    sig = lambda z: 1.0 / (1.0 + np.exp(-z))

    def step(x):
        hs, cs = [], []
        inp = x
        for (Wih, bih, bhh) in layers:
            g = inp @ Wih.T + bih + bhh
            i, f, gg, o = np.split(g, 4, axis=-1)
            c = sig(i) * np.tanh(gg)
            h = sig(o) * np.tanh(c)
            hs.append(h); cs.append(c); inp = h
        return inp.astype(np.float32), np.stack(hs).astype(np.float32), np.stack(cs).astype(np.float32)

    x0 = emb[inputs["inputs"][:, 0]]
    x1 = emb[inputs["inputs"][:, 1]]
    out0, h0, c0 = step(x0)
    out1, h1, c1 = step(x1)
    outputs = np.concatenate(
        [out0[None], np.broadcast_to(out1[None], (steps - 1, batch, H))], 0
    ).reshape(batch, steps, H)
    h_steps = np.concatenate(
        [h0, np.broadcast_to(h1[None], (steps - 1, L, batch, H)).reshape((steps - 1) * L, batch, H)], 0
    ).reshape(batch, steps, L * H)
    c_steps = np.concatenate(
        [c0, np.broadcast_to(c1[None], (steps - 1, L, batch, H)).reshape((steps - 1) * L, batch, H)], 0
    ).reshape(batch, steps, L * H)
    Wh = h_steps @ inputs["Whw"].T + inputs["Whb"]
    Wc = c_steps @ inputs["Wcw"].T + inputs["Wcb"]
    idx = np.arange(steps)[:, None] + np.arange(A)[None, :] - A
    valid = idx >= 0
    win = np.where(valid[None, :, :, None], Wh[:, np.clip(idx, 0, None)], 0.0)
    att = win + Wc[:, :, None, :]
    attn = att.mean(axis=2)
    concat_h = np.concatenate([attn, outputs], axis=2)
    outs = concat_h @ inputs["decw"].T + inputs["decb"]
    bi, ti = np.nonzero(np.arange(steps)[None, :] < (Ls[:, None] - 1))
    return outs[bi, ti].reshape(-1, V).astype(np.float32)


def _pack_inputs(inputs):
    import ml_dtypes
    f32 = np.float32
    bft = ml_dtypes.bfloat16
    emb = np.asarray(inputs["emb"], f32)
    idx0 = np.asarray(inputs["inputs"][:, 0]).astype(np.int64)
    idx1 = np.asarray(inputs["inputs"][:, 1]).astype(np.int64)

    def gates_pack(Wih):
        W = np.asarray(Wih, dtype=f32)
        return np.concatenate([W[0:H], W[2 * H:3 * H], W[3 * H:4 * H]], axis=0).T

    whb = np.asarray(inputs["Whb"], f32)
    wcb = np.asarray(inputs["Wcb"], f32)
    b0 = np.asarray(inputs["bih0"], f32) + np.asarray(inputs["bhh0"], f32)
    b1 = np.asarray(inputs["bih1"], f32) + np.asarray(inputs["bhh1"], f32)
    biasp = np.zeros((128, W1), f32)
    biasp[0:64, 0] = b0[0:H]
    biasp[64:128, 0] = b0[2 * H:3 * H]
    biasp[0:64, 1] = b0[3 * H:4 * H]
    biasp[0:64, 2] = b1[0:H]
    biasp[64:128, 2] = b1[2 * H:3 * H]
    biasp[0:64, 3] = b1[3 * H:4 * H]
    biasp[0:64, 4] = whb * (20.0 / A) + wcb

    CGh, CGc, C0h, C0c, cntfix = _build_count_matrices()
    # X partition p = 16m + q (slot-major); Sel maps class -> partition.
    # classes: 0=H 1=A 2=B (generic), 3=H' 4=A' 5=B' (batch-0, slot 0 only)
    Sel = np.zeros((6, 128), f32)
    for sp in range(128):
        q = sp % 16
        c = 0 if q == 0 else (1 if q % 2 == 1 else 2)
        if sp < 3:
            c = 3 + sp
        Sel[c, sp] = 1

    Whw = np.asarray(inputs["Whw"], f32)
    Wcw = np.asarray(inputs["Wcw"], f32)
    decw = np.asarray(inputs["decw"], f32)
    aux = np.zeros((128, W2), f32)
    aux[0:64, _XS:_XS + 64] = emb[idx0].T
    aux[0:64, _XS + 64:_XS + 128] = emb[idx1].T
    aux[0:64, _WIH0:_WIH0 + 192] = gates_pack(inputs["Wih0"])
    aux[0:64, _WIH1:_WIH1 + 192] = gates_pack(inputs["Wih1"])
    aux[0:64, _WHW:_WHW + 64] = Whw[:, 0:H].T / A
    aux[0:64, _WHW + 64:_WHW + 128] = Whw[:, H:2 * H].T / A
    aux[0:64, _WCW:_WCW + 64] = Wcw[:, 0:H].T
    aux[0:64, _WCW + 64:_WCW + 128] = Wcw[:, H:2 * H].T
    aux[:, _CGH:_CGH + 84] = CGh
    aux[:, _CGC:_CGC + 84] = CGc
    aux[0, _CNT:_CNT + 84] = cntfix
    aux[0, _WHBR:_WHBR + 64] = whb
    aux[0:6, _SEL:_SEL + 128] = Sel
    aux[0:64, _DECA:_DECA + 130] = decw[:, 0:H].T
    aux[0:64, _DECW2:_DECW2 + 130] = decw[:, H:2 * H].T
    aux[64, _DECW2:_DECW2 + 130] = np.asarray(inputs["decb"], f32)

    in_maps = []
    for core in range(NCORES):
        bp = biasp
        ax = aux.copy()
        if core == 0:
            bp = biasp.copy()
            bp[0:64, 6] = 1.0
            ax[:, _CBH:_CBH + 84] = C0h
            ax[:, _CBC:_CBC + 84] = C0c
        else:
            ax[:, _CBH:_CBH + 84] = CGh
            ax[:, _CBC:_CBC + 84] = CGc
        in_maps.append({"biasp": bp, "aux": ax.astype(bft)})
    return in_maps


def kernel(**inputs):
    inputs = {k: np.asarray(v) for k, v in inputs.items()}
    Ls = np.asarray(inputs["lengths"]).astype(np.int64)
    steps = int(Ls.max())
    if steps != S or inputs["inputs"].shape != (B, S):
        return _host_reference_fallback(inputs)

    from concourse.bass_utils import run_bass_kernel_spmd

    in_maps = _pack_inputs(inputs)
    nc = _get_nc()
    res = run_bass_kernel_spmd(nc, in_maps, core_ids=list(range(NCORES)))
    outs = np.concatenate(
        [r["out"].reshape(BPC, S, V) for r in res.results], axis=0)  # [64,512,130]

    bi, ti = np.nonzero(np.arange(steps)[None, :] < (Ls[:, None] - 1))
    return np.ascontiguousarray(outs[bi, ti].reshape(-1, V))


# revision 9
# speedup vs baseline: 1.1820x; 1.1820x over previous
"""Trainium2 Bass kernel for nn_Melody_RNN (B=64, S=512, A=20, V=130, E=H=64, L=2).

Structure exploited (all implied by the reference's exact semantics):
  * Only embedding rows for inputs[:,0] / inputs[:,1] are used; the LSTM runs
    exactly 2 timesteps (zero initial state, so the forget gate is dead).
  * The torch cat+view memory reinterpretations make every output row a
    function of s only: row s = OG[s] for s<84, OG[20+(s-84)%64] for s>=84,
    identical across batches except batch 0 rows 0..83 (OB table).
  * The attention-mask bug makes softmax exactly uniform, so the sliding
    window sum is a fixed linear map: attn = whTT^T @ Ch + wcTT^T @ Cc + bias
    with host-precomputed integer count matrices C.

Kernel v6 design:
  * bf16 LSTM batched over both timesteps (4 matmuls + 8 ACTs + 4 muls).
  * Window/attention collapsed into matmuls against host count matrices
    (bf16; counts are small ints, exact).
  * bf16 decode -> og/ob row tables [96,130] (rows 84:96 = generic rows
    20:32, produced by widening the decode matmul) -> DRAM staging
    (SBUF APs cannot fold partitions) -> Y [6,4160] = flattened 32-row
    classes (H=og[0:32], A=og[32:64], B=og[64:96]; H'/A'/B' from ob).
  * Selection matmul (bf16 0/1 weights, exact) broadcasts Y -> X [128,4160]
    in 512-col PSUM chunks; DVE/ACT copy to SBUF as f32.
  * X partition p = output row-block p (slot-major p = 16m+q), so the
    output is X flattened: 3 column-wave DMAs of 128 descriptors x 5-6KB
    each saturate all 16 SDMA engines near the HBM write roofline.

SPMD: 8 cores identical program; core 0's input pack carries the batch-0
count matrices and mvec=1 so its slot 0 blends to the special table.
"""

import sys
import numpy as np

if "/root/.axon_site/_ro/trn_rl_repo" not in sys.path:
    sys.path.insert(0, "/root/.axon_site/_ro/trn_rl_repo")

B, S, A = 64, 512, 20
V, E, H = 130, 64, 64
NCORES = 8
BPC = B // NCORES  # batches per core

SLOT = S * V          # 66560 elements per slot
RG = 32 * V           # 4160 elements per row-group / X partition

# aux bf16 pack columns
_XS = 0
_WIH0 = 128
_WIH1 = 320
_WHW = 512
_WCW = 640
_CBH = 768
_CBC = 852
_CGH = 936
_CGC = 1020
_CNT = 1104   # partition 0, [1, 84]
_WHBR = 1188  # partition 0, [1, 64]
_SEL = 1252   # partitions 0:6, [6, 128]
_DECA = 1380  # [64, 130]
_DECW2 = 1510  # [65, 130]
W2 = 1640
W1 = 8        # f32 bias pack cols

_NC_CACHE = {}


def _build_nc():
    import concourse.bass as bass
    import concourse.bacc as bacc
    import concourse.mybir as mybir
    from concourse.tile import TileContext

    f32 = mybir.dt.float32
    bf16 = mybir.dt.bfloat16
    AF = mybir.ActivationFunctionType

    nc = bacc.Bacc("TRN2", target_bir_lowering=False, debug=False)

    d_bias = nc.dram_tensor("biasp", [128, W1], f32, kind="ExternalInput")
    d_aux = nc.dram_tensor("aux", [128, W2], bf16, kind="ExternalInput")
    d_out = nc.dram_tensor("out", [BPC * S, V], f32, kind="ExternalOutput")
    d_fold = nc.dram_tensor("fold", [2, 96 * V], bf16, kind="Internal")

    with TileContext(nc) as tc:
        with (
            tc.tile_pool(name="sbuf", bufs=1) as pool,
            tc.tile_pool(name="psum", bufs=1, space="PSUM") as pp,
            tc.tile_pool(name="psx", bufs=3, space="PSUM") as px,
        ):
            # ---- input loads: LSTM-critical slice first ----
            t_aux = pool.tile([128, W2], bf16)
            t_bias = pool.tile([128, W1], f32)
            nc.sync.dma_start(out=t_aux[:, 0:512], in_=d_aux[:, 0:512])
            nc.scalar.dma_start(out=t_aux[:, 512:W2], in_=d_aux[:, 512:W2])
            nc.sync.dma_start(out=t_bias[:], in_=d_bias[:])

            xs = t_aux[0:64, _XS:_XS + 128]
            wih0 = t_aux[0:64, _WIH0:_WIH0 + 192]
            wih1 = t_aux[0:64, _WIH1:_WIH1 + 192]
            whw = t_aux[0:64, _WHW:_WHW + 128]
            wcw = t_aux[0:64, _WCW:_WCW + 128]
            BIA = t_bias

            ones = pool.tile([1, 4], f32)
            dummy = pool.tile([1, 2], f32)
            nc.vector.memset(ones[:], 1.0)
            # preload Sigmoid/Tanh ACT tables while input DMAs are in flight
            nc.scalar.activation(dummy[0:1, 0:1], ones[0:1, 0:1], AF.Sigmoid)
            nc.scalar.activation(dummy[0:1, 1:2], ones[0:1, 0:1], AF.Tanh)

            # ---- LSTM: both steps batched; hcat cols [l0s0|l0s1|l1s0|l1s1]
            hcat = pool.tile([H, 256], bf16)
            ccat = pool.tile([H, 256], bf16)

            def lstm_layer(rhsT, wp, bc, dst_off, tag):
                psg = pp.tile([128, 256], f32, tag="gates")
                nc.tensor.matmul(psg[:, 0:128], wp[:, 0:128], rhsT,
                                 start=True, stop=True)
                nc.tensor.matmul(psg[0:64, 128:256], wp[:, 128:192], rhsT,
                                 start=True, stop=True)
                sig_i = pool.tile([H, 128], f32, tag=f"sigi{tag}")
                tanh_g = pool.tile([H, 128], f32, tag=f"tanhg{tag}")
                sig_o = pool.tile([H, 128], f32, tag=f"sigo{tag}")
                tanh_c = pool.tile([H, 128], f32, tag=f"tanhc{tag}")
                nc.scalar.activation(tanh_g[:], psg[64:128, 0:128], AF.Tanh,
                                     bias=BIA[64:128, bc:bc + 1])
                nc.scalar.activation(sig_i[:], psg[0:64, 0:128], AF.Sigmoid,
                                     bias=BIA[0:64, bc:bc + 1])
                nc.scalar.activation(sig_o[:], psg[0:64, 128:256], AF.Sigmoid,
                                     bias=BIA[0:64, bc + 1:bc + 2])
                cc = ccat[:, dst_off:dst_off + 128]
                hh = hcat[:, dst_off:dst_off + 128]
                nc.vector.tensor_mul(cc, sig_i[:], tanh_g[:])
                nc.scalar.activation(tanh_c[:], cc, AF.Tanh)
                nc.vector.tensor_mul(hh, sig_o[:], tanh_c[:])

            lstm_layer(xs, wih0, 0, 0, "l0")
            lstm_layer(hcat[:, 0:128], wih1, 2, 128, "l1")
            out0T = hcat[:, 128:192]
            out1T = hcat[:, 192:256]

            # ---- outputs-half tiles [65, 96] bf16 (row 64 = ones) ----
            # cols 84:96 repeat generic rows 20:32 (periodic pad for the fold)
            outG = pool.tile([65, 96], bf16)
            outB = pool.tile([65, 96], bf16)
            d64 = pool.tile([64, 64], f32)
            MV = BIA[0:64, 6:7]
            nc.vector.tensor_sub(d64[:], out0T, out1T)
            nc.vector.tensor_scalar_mul(d64[:], d64[:], MV)
            nc.vector.tensor_add(outB[0:64, 0:64], out1T, d64[:])
            nc.vector.tensor_copy(outB[0:64, 64:96], out1T[:, 0:32])
            nc.vector.memset(outB[64:65, :], 1.0)
            nc.gpsimd.tensor_copy(outG[0:64, 0:64], out1T)
            nc.gpsimd.tensor_copy(outG[0:64, 64:96], out1T[:, 0:32])
            nc.vector.memset(outG[64:65, :], 1.0)

            # ---- mm1: whTT/wcTT [128, 64] (bf16 copies for mm2) ----
            pstab = pp.tile([128, 128], f32, tag="tab")
            whTTp = pstab[:, 0:64]
            wcTTp = pstab[:, 64:128]
            nc.tensor.matmul(whTTp, hcat[:, 0:256:2], whw[:, 0:64],
                             start=True, stop=False)
            nc.tensor.matmul(whTTp, hcat[:, 1:256:2], whw[:, 64:128],
                             start=False, stop=True)
            nc.tensor.matmul(wcTTp, ccat[:, 0:256:2], wcw[:, 0:64],
                             start=True, stop=False)
            nc.tensor.matmul(wcTTp, ccat[:, 1:256:2], wcw[:, 64:128],
                             start=False, stop=True)
            whTT = pool.tile([128, 64], bf16)
            wcTT = pool.tile([128, 64], bf16)
            nc.scalar.copy(whTT[:], whTTp)
            nc.vector.tensor_copy(wcTT[:], wcTTp)

            # ---- mm2: attn [64, 84] = whTT^T@Ch + wcTT^T@Cc + whb x cntfix
            CBh = t_aux[:, _CBH:_CBH + 84]
            CBc = t_aux[:, _CBC:_CBC + 84]
            CGh = t_aux[:, _CGH:_CGH + 84]
            CGc = t_aux[:, _CGC:_CGC + 84]
            CNT = t_aux[0:1, _CNT:_CNT + 84]
            WHBR = t_aux[0:1, _WHBR:_WHBR + 64]
            ABIAS = BIA[0:64, 4:5]
            psat = pp.tile([64, 168], f32, tag="attn")

            def attn_mm(Ch, Cc, c0):
                ps = psat[:, c0:c0 + 84]
                nc.tensor.matmul(ps, whTT[:], Ch, start=True, stop=False)
                nc.tensor.matmul(ps, wcTT[:], Cc, start=False, stop=False)
                nc.tensor.matmul(ps, WHBR, CNT, start=False, stop=True)
                return ps

            attnBp = attn_mm(CBh, CBc, 84)
            attnGp = attn_mm(CGh, CGc, 0)
            attnG = pool.tile([64, 96], bf16)
            attnB = pool.tile([64, 96], bf16)
            nc.vector.tensor_scalar_add(attnB[:, 0:84], attnBp, ABIAS)
            nc.vector.tensor_scalar_add(attnB[:, 84:96], psat[0:64, 20:32], ABIAS)
            nc.scalar.activation(attnG[:, 0:84], attnGp, AF.Identity, bias=ABIAS)
            nc.scalar.activation(attnG[:, 84:96], psat[0:64, 20:32],
                                 AF.Identity, bias=ABIAS)

            # ---- decode: ob first (its fold gates the output pipeline) ----
            DECA = t_aux[0:64, _DECA:_DECA + 130]
            DECW2 = t_aux[0:65, _DECW2:_DECW2 + 130]
            psdec = pp.tile([96, 2 * V], f32, tag="dec")
            obPp = psdec[:, 0:V]
            ogPp = psdec[:, V:2 * V]
            nc.tensor.matmul(obPp, outB[:], DECW2, start=True, stop=False)
            nc.tensor.matmul(obPp, attnB[:], DECA, start=False, stop=True)
            nc.tensor.matmul(ogPp, outG[:], DECW2, start=True, stop=False)
            nc.tensor.matmul(ogPp, attnG[:], DECA, start=False, stop=True)

            og96 = pool.tile([96, V], bf16)
            ob96 = pool.tile([96, V], bf16)
            nc.vector.tensor_copy(ob96[:], obPp)
            nc.scalar.copy(og96[:], ogPp)

            # ---- fold via DRAM: og96/ob96 -> d_fold -> Y [6, 4160] ----
            # (SBUF APs may only cross partitions in dim 0, so a direct
            # SBUF->SBUF fold is not expressible; DRAM staging is.)
            Y = pool.tile([6, RG], bf16)
            Yt = Y[:].tensor
            nc.sync.dma_start(out=d_fold[1:2, :], in_=ob96[:])
            nc.scalar.dma_start(out=d_fold[0:1, :], in_=og96[:])
            nc.sync.dma_start(
                out=bass.AP(Yt, 0, [[RG, 6], [1, 2080]]),
                in_=bass.AP(d_fold, 0, [[RG, 6], [1, 2080]]))
            nc.scalar.dma_start(
                out=bass.AP(Yt, 2080, [[RG, 6], [1, 2080]]),
                in_=bass.AP(d_fold, 2080, [[RG, 6], [1, 2080]]))

            # ---- X broadcast: Sel [6,128] x Y chunks -> X [128, 4160] ----
            SEL = t_aux[0:6, _SEL:_SEL + 128]
            X = pool.tile([128, RG], f32)
            Xt = X[:].tensor
            copy_eng = [nc.vector, nc.scalar]
            CHUNKS = [(n * 512, min(512, RG - n * 512)) for n in range(9)]

            def x_chunks(rng):
                for n in rng:
                    c0, cw = CHUNKS[n]
                    ps = px.tile([128, 512], f32, tag="xch")
                    nc.tensor.matmul(ps[:, 0:cw], SEL, Y[:, c0:c0 + cw],
                                     start=True, stop=True)
                    eng = copy_eng[n % 2]
                    if eng is nc.vector:
                        eng.tensor_copy(X[:, c0:c0 + cw], ps[:, 0:cw])
                    else:
                        eng.copy(X[:, c0:c0 + cw], ps[:, 0:cw])

            def out_wave(col0, cw):
                # X partition p = output row-block p: pure partition-major
                nc.gpsimd.dma_start(
                    out=bass.AP(d_out, col0, [[RG, 128], [1, cw]]),
                    in_=bass.AP(Xt, col0, [[RG, 128], [1, cw]]))

            x_chunks(range(0, 3))
            out_wave(0, 1536)
            x_chunks(range(3, 6))
            out_wave(1536, 1536)
            x_chunks(range(6, 9))
            out_wave(3072, RG - 3072)

    nc.compile()
    return nc


def _get_nc():
    if "nc" not in _NC_CACHE:
        _NC_CACHE["nc"] = _build_nc()
    return _NC_CACHE["nc"]


def _build_count_matrices():
    colmapG = list(range(32, 64)) + list(range(96, 128)) + list(range(32, 51))
    colmap0 = list(range(0, 32)) + list(range(64, 96)) + list(range(32, 51))
    wccolG = list(range(32, 64)) + list(range(96, 128)) + list(range(32, 52))
    wccol0 = list(range(0, 32)) + list(range(64, 96)) + list(range(32, 52))

    def ch(colmap):
        C = np.zeros((128, 84), np.float32)
        cnt = np.zeros(84, np.float32)
        for r in range(84):
            for j in range(max(r, 20), r + 20):
                C[colmap[j - 20], r] += 1.0
                cnt[r] += 1.0
        return C, cnt

    def cc(wccol):
        C = np.zeros((128, 84), np.float32)
        for r in range(84):
            C[wccol[r], r] += 1.0
        return C

    CGh, cnt = ch(colmapG)
    C0h, _ = ch(colmap0)
    cntfix = (cnt - 20.0) / A
    return CGh, cc(wccolG), C0h, cc(wccol0), cntfix


def _host_reference_fallback(inputs):
    """Pure-numpy replica of the reference for steps != 512 (never hit with the
    canonical setup_inputs, which fixes lengths = 512)."""
    emb = inputs["emb"]; L = 2
    Ls = np.asarray(inputs["lengths"]); steps = int(Ls.max()); batch = inputs["inputs"].shape[0]
    layers = [(inputs["Wih0"], inputs["bih0"], inputs["bhh0"]),
              (inputs["Wih1"], inputs["bih1"], inputs["bhh1"])]
    sig = lambda z: 1.0 / (1.0 + np.exp(-z))

    def step(x):
        hs, cs = [], []
        inp = x
        for (Wih, bih, bhh) in layers:
            g = inp @ Wih.T + bih + bhh
            i, f, gg, o = np.split(g, 4, axis=-1)
            c = sig(i) * np.tanh(gg)
            h = sig(o) * np.tanh(c)
            hs.append(h); cs.append(c); inp = h
        return inp.astype(np.float32), np.stack(hs).astype(np.float32), np.stack(cs).astype(np.float32)

    x0 = emb[inputs["inputs"][:, 0]]
    x1 = emb[inputs["inputs"][:, 1]]
    out0, h0, c0 = step(x0)
    out1, h1, c1 = step(x1)
    outputs = np.concatenate(
        [out0[None], np.broadcast_to(out1[None], (steps - 1, batch, H))], 0
    ).reshape(batch, steps, H)
    h_steps = np.concatenate(
        [h0, np.broadcast_to(h1[None], (steps - 1, L, batch, H)).reshape((steps - 1) * L, batch, H)], 0
    ).reshape(batch, steps, L * H)
    c_steps = np.concatenate(
        [c0, np.broadcast_to(c1[None], (steps - 1, L, batch, H)).reshape((steps - 1) * L, batch, H)], 0
    ).reshape(batch, steps, L * H)
    Wh = h_steps @ inputs["Whw"].T + inputs["Whb"]
    Wc = c_steps @ inputs["Wcw"].T + inputs["Wcb"]
    idx = np.arange(steps)[:, None] + np.arange(A)[None, :] - A
    valid = idx >= 0
    win = np.where(valid[None, :, :, None], Wh[:, np.clip(idx, 0, None)], 0.0)
    att = win + Wc[:, :, None, :]
    attn = att.mean(axis=2)
    concat_h = np.concatenate([attn, outputs], axis=2)
    outs = concat_h @ inputs["decw"].T + inputs["decb"]
    bi, ti = np.nonzero(np.arange(steps)[None, :] < (Ls[:, None] - 1))
    return outs[bi, ti].reshape(-1, V).astype(np.float32)


def _pack_inputs(inputs):
    import ml_dtypes
    f32 = np.float32
    bft = ml_dtypes.bfloat16
    emb = np.asarray(inputs["emb"], f32)
    idx0 = np.asarray(inputs["inputs"][:, 0]).astype(np.int64)
    idx1 = np.asarray(inputs["inputs"][:, 1]).astype(np.int64)

    def gates_pack(Wih):
        W = np.asarray(Wih, dtype=f32)
        return np.concatenate([W[0:H], W[2 * H:3 * H], W[3 * H:4 * H]], axis=0).T

    whb = np.asarray(inputs["Whb"], f32)
    wcb = np.asarray(inputs["Wcb"], f32)
    b0 = np.asarray(inputs["bih0"], f32) + np.asarray(inputs["bhh0"], f32)
    b1 = np.asarray(inputs["bih1"], f32) + np.asarray(inputs["bhh1"], f32)
    biasp = np.zeros((128, W1), f32)
    biasp[0:64, 0] = b0[0:H]
    biasp[64:128, 0] = b0[2 * H:3 * H]
    biasp[0:64, 1] = b0[3 * H:4 * H]
    biasp[0:64, 2] = b1[0:H]
    biasp[64:128, 2] = b1[2 * H:3 * H]
    biasp[0:64, 3] = b1[3 * H:4 * H]
    biasp[0:64, 4] = whb * (20.0 / A) + wcb

    CGh, CGc, C0h, C0c, cntfix = _build_count_matrices()
    # X partition p = 16m + q (slot-major); Sel maps class -> partition.
    # classes: 0=H 1=A 2=B (generic), 3=H' 4=A' 5=B' (batch-0, slot 0 only)
    Sel = np.zeros((6, 128), f32)
    for sp in range(128):
        q = sp % 16
        c = 0 if q == 0 else (1 if q % 2 == 1 else 2)
        if sp < 3:
            c = 3 + sp
        Sel[c, sp] = 1

    Whw = np.asarray(inputs["Whw"], f32)
    Wcw = np.asarray(inputs["Wcw"], f32)
    decw = np.asarray(inputs["decw"], f32)
    aux = np.zeros((128, W2), f32)
    aux[0:64, _XS:_XS + 64] = emb[idx0].T
    aux[0:64, _XS + 64:_XS + 128] = emb[idx1].T
    aux[0:64, _WIH0:_WIH0 + 192] = gates_pack(inputs["Wih0"])
    aux[0:64, _WIH1:_WIH1 + 192] = gates_pack(inputs["Wih1"])
    aux[0:64, _WHW:_WHW + 64] = Whw[:, 0:H].T / A
    aux[0:64, _WHW + 64:_WHW + 128] = Whw[:, H:2 * H].T / A
    aux[0:64, _WCW:_WCW + 64] = Wcw[:, 0:H].T
    aux[0:64, _WCW + 64:_WCW + 128] = Wcw[:, H:2 * H].T
    aux[:, _CGH:_CGH + 84] = CGh
    aux[:, _CGC:_CGC + 84] = CGc
    aux[0, _CNT:_CNT + 84] = cntfix
    aux[0, _WHBR:_WHBR + 64] = whb
    aux[0:6, _SEL:_SEL + 128] = Sel
    aux[0:64, _DECA:_DECA + 130] = decw[:, 0:H].T
    aux[0:64, _DECW2:_DECW2 + 130] = decw[:, H:2 * H].T
    aux[64, _DECW2:_DECW2 + 130] = np.asarray(inputs["decb"], f32)

    in_maps = []
    for core in range(NCORES):
        bp = biasp
        ax = aux.copy()
        if core == 0:
            bp = biasp.copy()
            bp[0:64, 6] = 1.0
            ax[:, _CBH:_CBH + 84] = C0h
            ax[:, _CBC:_CBC + 84] = C0c
        else:
            ax[:, _CBH:_CBH + 84] = CGh
            ax[:, _CBC:_CBC + 84] = CGc
        in_maps.append({"biasp": bp, "aux": ax.astype(bft)})
    return in_maps


def kernel(**inputs):
    inputs = {k: np.asarray(v) for k, v in inputs.items()}
    Ls = np.asarray(inputs["lengths"]).astype(np.int64)
    steps = int(Ls.max())
    if steps != S or inputs["inputs"].shape != (B, S):
        return _host_reference_fallback(inputs)

    from concourse.bass_utils import run_bass_kernel_spmd

    in_maps = _pack_inputs(inputs)
    nc = _get_nc()
    res = run_bass_kernel_spmd(nc, in_maps, core_ids=list(range(NCORES)))
    outs = np.concatenate(
        [r["out"].reshape(BPC, S, V) for r in res.results], axis=0)  # [64,512,130]

    bi, ti = np.nonzero(np.arange(steps)[None, :] < (Ls[:, None] - 1))
    return np.ascontiguousarray(outs[bi, ti].reshape(-1, V))
